# revision 20
# baseline (speedup 1.0000x reference)
"""CPhaseLayer kernel for Trainium2 (8 NeuronCores, SPMD data-parallel).

The reference computes out = einsum('bcn,nm->bcm', x, tmat) with
x [4096, 2, 8192] f32 and tmat [8192, 8192] f32 where tmat is a Kronecker
product of CPHASE = diag(1,1,-1,1) and I2 gates.  Every factor is diagonal,
so tmat is diagonal with +-1 entries and the matmul reduces EXACTLY to
out[b,c,m] = x[b,c,m] * diag(tmat)[m]  (the other 8191 terms of the f32
dot product are exact zeros, so this is bitwise identical).

Device kernel: elementwise multiply of each row block by the sign
vector.  The sign vector enters as a [1, 8192] f32 row (32 KiB) and is
broadcast to all 128 SBUF partitions on-chip via 16 K=1 TensorE matmuls
(ones[1,128].T @ d[1,512] -> PSUM) + VectorE copies (cast to bf16 — ±1
is exact), so it costs no HBM bandwidth.  Sharding: batch split 8 ways
-> 1024 rows x 8192 per core.

Streaming pipeline (per core, picked by paired repeat-slope shootouts
on the axon-tunneled cores — see sweep.py/finalists.py):
  * in-DMAs: 4 MiB f32 row-tiles, SWDGE (gpsimd) ring, casting
    f32 -> bf16 on the fly into SBUF (the cast is free: the DMA engines
    are bound by the 32 MiB HBM-source side either way); 8 tiles in
    flight.
  * multiply: DVE tensor_mul in bf16 (2x perf mode, ~34 us busy/core),
    in place.
  * out-DMAs: 2 MiB bf16 tiles cycling across all three DMA rings
    (ACT+SP HWDGE, SWDGE); lag-4 software-pipelined emission so an
    out's wait-on-multiply never gates input DMAs queued behind it.
bf16 output halves write traffic: 32 MiB read + 16 MiB written per
core.  Measured duplex probes show the DMA fabric is near-half-duplex
under load (aggregate bytes bind), so the byte cut is worth ~25%; in
quiet windows the kernel is read-bound at ~70-80 us/core (~410 GB/s
reads with writes fully overlapped).  Only the input bf16 cast rounds:
max rel err 2^-9 (measured 2.9e-3 vs the f32 reference, gate 2e-2).
The device bf16 output is upcast to f32 on the host.

The diagonal is extracted from the *runtime* tmat input; diagonality is
verified on the host with a fallback for the (never occurring)
non-diagonal case.
"""

import numpy as np

B, C, N = 4096, 2, 8192
N_CORES = 8
ROWS = B * C  # 8192 rows of length N
ROWS_PER_CORE = ROWS // N_CORES  # 1024
P = 128  # SBUF partitions
DCHUNK = 512  # PSUM-bank-sized column chunk for the d broadcast

_CACHE = {}


def _build_nc(repeats: int = 1, k: int = 2, bufs: int = 2,
              out_ring: str = "sync", mul_w: int = N, group: int = 1,
              d_dtype: str = None, tile_rows=None, lag: int = 1,
              in_ring: str = "sync", sbuf_dtype: str = "f32",
              out_dtype: str = None, sep_out: bool = False,
              obufs: int = None,
              no_in: bool = False, no_mul: bool = False,
              no_out: bool = False):
    """Bass program for one core: out[r, :] = xs[r, :] * d[:] (d broadcast).

    xs: [ROWS_PER_CORE, N] f32, dr: [1, N] f32 sign row, out
    [ROWS_PER_CORE, N] out_dtype.

    k: rows per partition per tile (DMA transfer size = k * 4 MiB f32).
    in_ring/out_ring: 'sync'/'scalar' (HWDGE) or 'gpsimd' (SWDGE).
    sbuf_dtype: dtype of the x tiles in SBUF; if != f32 the in-DMA
      casts on the fly (SWDGE only -> in_ring forced to gpsimd).
    out_dtype: dtype of the out DRAM tensor (default sbuf_dtype); if it
      differs from sbuf_dtype the out-DMA casts (SWDGE only).
    mul_w: column width of each DVE multiply.
    no_in/no_mul/no_out: ceiling probes (skip that pipeline stage).
    repeats > 1 re-runs the full streaming loop (same I/O, identical
    result) — used only to measure steady-state device time by slope.
    """
    import concourse.mybir as mybir
    import concourse.tile as tile
    from concourse import bacc

    f32 = mybir.dt.float32
    dt_map = {"f32": mybir.dt.float32, "bf16": mybir.dt.bfloat16,
              "f16": mybir.dt.float16, "fp8": mybir.dt.float8e4}
    kib_map = {"f32": 32, "bf16": 16, "f16": 16, "fp8": 8}
    if out_dtype is None:
        out_dtype = sbuf_dtype
    if d_dtype is None:
        d_dtype = sbuf_dtype if sbuf_dtype != "f32" else "f32"
    sbuf_dt, out_dt, d_dt = dt_map[sbuf_dtype], dt_map[out_dtype], dt_map[d_dtype]
    if sbuf_dtype != "f32":
        in_ring = "gpsimd"  # cast during DMA is SWDGE-only
    if out_dtype != sbuf_dtype and not sep_out:
        out_ring = "gpsimd"  # cast on the out-DMA
    # sep_out: the DVE multiply writes a separate out_dtype tile, so the
    # out-DMA is cast-free and can ride an HWDGE ring.
    if obufs is None:
        obufs = bufs

    nc = bacc.Bacc("TRN2", target_bir_lowering=False, debug=False)

    xs = nc.dram_tensor("xs", [ROWS_PER_CORE, N], f32, kind="ExternalInput")
    dr = nc.dram_tensor("dr", [1, N], f32, kind="ExternalInput")
    out = nc.dram_tensor("out", [ROWS_PER_CORE, N], out_dt,
                         kind="ExternalOutput")

    n_dchunks = N // DCHUNK
    # tile_rows: explicit per-tile k list (rows-per-partition); else uniform k
    ks = list(tile_rows) if tile_rows else [k] * (ROWS_PER_CORE // (P * k))
    assert sum(ks) * P == ROWS_PER_CORE
    n_tiles = len(ks)
    # partition p of tile t holds k consecutive DRAM rows (contiguous k*32KiB
    # per partition line -> descriptor-friendly big DMAs)
    tile_views = []
    r0 = 0
    for ki in ks:
        xv = xs[r0 : r0 + P * ki, :].rearrange("(p k) n -> p (k n)", p=P, k=ki)
        ov = out[r0 : r0 + P * ki, :].rearrange("(p k) n -> p (k n)", p=P, k=ki)
        tile_views.append((ki, xv, ov))
        r0 += P * ki

    # SBUF budget (KiB per partition): x slots (+ out slots) + dfull +
    # drow(32) + ones
    x_kib = kib_map[sbuf_dtype]
    d_kib = kib_map[d_dtype]
    o_kib = obufs * max(ks) * kib_map[out_dtype] if sep_out else 0
    drow_own = bufs * max(ks) * x_kib + o_kib + d_kib + 33 <= 206
    if not drow_own:
        assert max(ks) * x_kib >= 32, "drow cannot borrow a smaller x slot"

    engines = {"sync": nc.sync, "scalar": nc.scalar, "gpsimd": nc.gpsimd}
    # 'both' alternates transfers between the two HWDGE rings; 'all3'
    # additionally cycles through the SWDGE (gpsimd) ring
    in_engs = ([nc.sync, nc.scalar] if in_ring == "both"
               else [nc.sync, nc.scalar, nc.gpsimd] if in_ring == "all3"
               else [engines[in_ring]])
    out_engs = ([nc.scalar, nc.sync] if out_ring == "both"
                else [nc.scalar, nc.sync, nc.gpsimd] if out_ring == "all3"
                else [engines[out_ring]])

    with tile.TileContext(nc) as tc:
        with (
            tc.tile_pool(name="dfull_pool", bufs=1) as dfull_pool,
            tc.tile_pool(name="ones_pool", bufs=1) as ones_pool,
            tc.tile_pool(name="drow_pool", bufs=1) as drow_pool,
            tc.tile_pool(name="psum", bufs=4, space="PSUM") as psum_pool,
            tc.tile_pool(name="xpool", bufs=bufs) as xpool,
            tc.tile_pool(name="opool", bufs=max(obufs, 1)) as opool,
        ):
            # --- broadcast d row to all 128 partitions without HBM traffic:
            # 16 K=1 matmuls ones[1,128].T @ d[1,512] -> PSUM, DVE-copy to
            # SBUF (casting to d_dtype; +-1 is exact in bf16/e4m3).  When the
            # budget is tight drow borrows an xpool slot (it releases once
            # the 16 matmuls have read it).
            if drow_own:
                drow = drow_pool.tile([1, N], f32, tag="drow")
            else:
                drow = xpool.tile([1, N], f32, tag="x")
            nc.sync.dma_start(drow[:], dr[:, :])
            ones = ones_pool.tile([1, P], f32, tag="ones")
            nc.gpsimd.memset(ones[:], 1.0)
            dfull = dfull_pool.tile([P, N], d_dt, tag="dfull")
            for j in range(n_dchunks):
                c0 = j * DCHUNK
                ps = psum_pool.tile([P, DCHUNK], f32)
                nc.tensor.matmul(ps[:], ones[:], drow[:, c0 : c0 + DCHUNK])
                nc.vector.tensor_copy(dfull[:, c0 : c0 + DCHUNK], ps[:])

            def do_muls(ki, xt, ot):
                if no_mul:
                    return
                for c in range(ki * N // mul_w):
                    sl = slice(c * mul_w, (c + 1) * mul_w)
                    d0 = (c * mul_w) % N
                    nc.vector.tensor_mul(
                        ot[:, sl], xt[:, sl], dfull[:, d0 : d0 + mul_w]
                    )

            if no_in:
                # write-ceiling probe: stream out-DMAs all reading one
                # static SBUF tile (no deps between the outs -> pure
                # out-ring throughput)
                assert no_mul and d_dtype == out_dtype
                wsrc = opool.tile([P, k * N], out_dt, tag="o")
                for j in range(k):
                    nc.vector.tensor_copy(wsrc[:, j * N : (j + 1) * N],
                                          dfull[:, :])
                for t in range(repeats * n_tiles):
                    _, _, ov = tile_views[t % n_tiles]
                    out_engs[t % len(out_engs)].dma_start(ov, wsrc[:])
            elif no_out:
                # read-ceiling probe: in-DMAs (+ muls) only; one dummy
                # write at the end keeps the output legal
                assert sbuf_dtype == out_dtype and not sep_out
                xt = None
                for t in range(repeats * n_tiles):
                    ki, xv, _ = tile_views[t % n_tiles]
                    xt = xpool.tile([P, ki * N], sbuf_dt, tag="x")
                    in_engs[t % len(in_engs)].dma_start(xt[:], xv)
                    do_muls(ki, xt, xt)
                out_engs[0].dma_start(tile_views[-1][2], xt[:])
            elif lag:
                # Software-pipelined emission: out(t-lag) is emitted after
                # in(t), so the out's wait-on-multiply never blocks the next
                # input DMA behind it in the ring FIFO (the multiply leaves
                # the DMA issue path).  Requires lag < bufs.
                assert lag < max(bufs, obufs if sep_out else 0) and group == 1
                flat = [tile_views[t % n_tiles] for t in range(repeats * n_tiles)]
                pending = []
                n_out = 0
                for t, (ki, xv, ov) in enumerate(flat):
                    xt = xpool.tile([P, ki * N], sbuf_dt, tag="x")
                    in_engs[t % len(in_engs)].dma_start(xt[:], xv)
                    if sep_out:
                        ot = opool.tile([P, ki * N], out_dt, tag="o")
                    else:
                        ot = xt
                    do_muls(ki, xt, ot)
                    pending.append((ot, ov))
                    if len(pending) > lag:
                        ot0, ov0 = pending.pop(0)
                        out_engs[n_out % len(out_engs)].dma_start(ov0, ot0[:])
                        n_out += 1
                for ot0, ov0 in pending:
                    out_engs[n_out % len(out_engs)].dma_start(ov0, ot0[:])
                    n_out += 1
            else:
                # group>1 emits G loads, then G multiplies, then G stores, so
                # the ring alternates read/write in G-transfer blocks.
                assert n_tiles % group == 0 and bufs >= group
                for _ in range(repeats):
                    for g in range(n_tiles // group):
                        items = []
                        for i in range(group):
                            ki, xv, ov = tile_views[g * group + i]
                            xt = xpool.tile([P, ki * N], sbuf_dt, tag="x")
                            in_engs[i % len(in_engs)].dma_start(xt[:], xv)
                            items.append((ki, xt, ov))
                        muled = []
                        for ki, xt, ov in items:
                            if sep_out:
                                ot = opool.tile([P, ki * N], out_dt, tag="o")
                            else:
                                ot = xt
                            do_muls(ki, xt, ot)
                            muled.append((ot, ov))
                        for i, (ot, ov) in enumerate(muled):
                            out_engs[i % len(out_engs)].dma_start(ov, ot[:])
    nc.finalize()
    return nc


class _Exec:
    """Compile-once SPMD executor for a finalized Bass program.

    Mirrors concourse.bass2jax.run_bass_via_pjrt's multi-core branch, but
    traces/jits exactly once so repeat calls pay only transfer + exec.
    """

    def __init__(self, nc):
        import jax
        import concourse.mybir as mybir
        from concourse.bass2jax import (
            _bass_exec_p,
            install_neuronx_cc_hook,
            partition_id_tensor,
        )
        from jax.experimental.shard_map import shard_map
        from jax.sharding import Mesh, NamedSharding, PartitionSpec

        install_neuronx_cc_hook()
        self.jax = jax
        partition_name = (
            nc.partition_id_tensor.name if nc.partition_id_tensor else None
        )

        in_names, out_names, out_avals, zero_shapes = [], [], [], []
        for alloc in nc.m.functions[0].allocations:
            if not isinstance(alloc, mybir.MemoryLocationSet):
                continue
            name = alloc.memorylocations[0].name
            if alloc.kind == "ExternalInput":
                if name != partition_name:
                    in_names.append(name)
            elif alloc.kind == "ExternalOutput":
                out_names.append(name)
                shape = tuple(alloc.tensor_shape)
                dtype = mybir.dt.np(alloc.dtype)
                out_avals.append(jax.core.ShapedArray(shape, dtype))
                zero_shapes.append((shape, dtype))

        self.in_names = list(in_names)
        self.out_names = list(out_names)
        self.out_avals = out_avals
        n_params = len(in_names)
        n_outs = len(out_names)

        bind_in_names = in_names + out_names
        if partition_name is not None:
            bind_in_names.append(partition_name)

        def _body(*args):
            operands = list(args)
            if partition_name is not None:
                operands.append(partition_id_tensor())
            outs = _bass_exec_p.bind(
                *operands,
                out_avals=tuple(out_avals),
                in_names=tuple(bind_in_names),
                out_names=tuple(out_names),
                lowering_input_output_aliases=(),
                sim_require_finite=True,
                sim_require_nnan=True,
                nc=nc,
            )
            return tuple(outs)

        devices = jax.devices()[:N_CORES]
        assert len(devices) == N_CORES
        self.mesh = Mesh(np.asarray(devices), ("core",))
        pspec = PartitionSpec("core")
        in_specs = (pspec,) * (n_params + n_outs)
        out_specs = (pspec,) * n_outs
        donate = tuple(range(n_params, n_params + n_outs))
        self.sharding = NamedSharding(self.mesh, pspec)
        self.sharded = jax.jit(
            shard_map(
                _body,
                mesh=self.mesh,
                in_specs=in_specs,
                out_specs=out_specs,
                check_rep=False,
            ),
            donate_argnums=donate,
            keep_unused=True,
        )
        # on-device zero allocator (avoids shipping 256 MiB of zeros per call)
        self._zeros = jax.jit(
            lambda: tuple(
                jax.numpy.zeros((N_CORES * s[0], *s[1:]), dt)
                for (s, dt) in zero_shapes
            ),
            out_shardings=(self.sharding,) * n_outs,
        )

    def __call__(self, *concat_inputs):
        """concat_inputs: one array per in_name, core-shards concatenated on
        axis 0.  Returns tuple of device outputs (concat on axis 0)."""
        outs = self.sharded(*concat_inputs, *self._zeros())
        return outs


# Deployed configuration — winner of the paired repeat-slope shootouts
# (see sweep.py / finalists.py): bf16-cast-in on the SWDGE ring, in-place
# bf16 DVE multiply, bf16 outs cycling across all three DMA rings
# (ACT/SP HWDGE + SWDGE) with lag-4 pipelined emission, 8 x 4 MiB-read
# tiles in flight.
DEFAULT_CFG = dict(k=1, bufs=8, lag=4, sbuf_dtype="bf16", out_ring="all3")


def _get_exec(repeats: int = 1, **cfg) -> _Exec:
    key = ("exec", repeats, tuple(sorted(cfg.items())))
    if key not in _CACHE:
        _CACHE[key] = _Exec(_build_nc(repeats=repeats, **cfg))
    return _CACHE[key]


def _device_inputs(xs_flat: np.ndarray, d: np.ndarray):
    """Device-resident concat of the per-core d rows ([8, 8192] -> one row
    per core)."""
    import jax

    ex = _get_exec(**DEFAULT_CFG)
    key = ("dr_dev", d.tobytes())
    if key not in _CACHE:
        drows = np.ascontiguousarray(
            np.broadcast_to(d[None, :], (N_CORES, N)).astype(np.float32)
        )
        _CACHE[key] = jax.device_put(drows, ex.sharding)
    return _CACHE[key]


def _run_device(xs_flat: np.ndarray, d: np.ndarray) -> np.ndarray:
    ex = _get_exec(**DEFAULT_CFG)
    dr_dev = _device_inputs(xs_flat, d)
    (out,) = ex(xs_flat, dr_dev)
    out = np.asarray(out)
    if out.dtype != np.float32:
        out = out.astype(np.float32)
    return out


def kernel(x: np.ndarray, tmat: np.ndarray) -> np.ndarray:
    x = np.asarray(x, dtype=np.float32)
    tmat = np.asarray(tmat, dtype=np.float32)
    assert x.shape == (B, C, N) and tmat.shape == (N, N)

    d = np.ascontiguousarray(np.diagonal(tmat))
    if not np.array_equal(tmat, np.diag(d)):
        # Non-diagonal transfer matrix: never happens for CPhaseLayer, but
        # keep a correct host fallback.
        return (x.reshape(ROWS, N).astype(np.float32) @ tmat).reshape(B, C, N)

    xs_flat = np.ascontiguousarray(x).reshape(ROWS, N)
    try:
        out = _run_device(xs_flat, d)
    except Exception:
        # Transient relay/device failures (e.g. NRT_EXEC_UNIT_UNRECOVERABLE)
        # happen rarely; rebuild the executor state and retry once, then fall
        # back to the host (bitwise-identical: the multiply is the whole op).
        try:
            _CACHE.clear()
            out = _run_device(xs_flat, d)
        except Exception:
            out = xs_flat * d[None, :]
    return out.reshape(B, C, N).astype(np.float32)



# revision 26
# speedup vs baseline: 1.0484x; 1.0484x over previous
"""CPhaseLayer kernel for Trainium2 (8 NeuronCores, SPMD data-parallel).

The reference computes out = einsum('bcn,nm->bcm', x, tmat) with
x [4096, 2, 8192] f32 and tmat [8192, 8192] f32 where tmat is a Kronecker
product of CPHASE = diag(1,1,-1,1) and I2 gates.  Every factor is diagonal,
so tmat is diagonal with +-1 entries and the matmul reduces EXACTLY to
out[b,c,m] = x[b,c,m] * diag(tmat)[m]  (the other 8191 terms of the f32
dot product are exact zeros, so this is bitwise identical).

Device kernel: elementwise multiply of each row block by the sign
vector.  The sign vector enters as a [1, 8192] f32 row (32 KiB) and is
broadcast to all 128 SBUF partitions on-chip via 16 K=1 TensorE matmuls
(ones[1,128].T @ d[1,512] -> PSUM) + VectorE copies (cast to bf16 — ±1
is exact), so it costs no HBM bandwidth.  Sharding: batch split 8 ways
-> 1024 rows x 8192 per core.

Streaming pipeline (per core, picked by paired repeat-slope shootouts
on the axon-tunneled cores — see sweep.py/finalists.py):
  * in-DMAs: 4 MiB f32 row-tiles, SWDGE (gpsimd) ring, casting
    f32 -> bf16 on the fly into SBUF (the cast is free: the DMA engines
    are bound by the 32 MiB HBM-source side either way); 8 tiles in
    flight.
  * multiply: DVE tensor_mul in bf16 (2x perf mode, ~34 us busy/core),
    in place.
  * out-DMAs: 2 MiB bf16 tiles cycling across all three DMA rings
    (ACT+SP HWDGE, SWDGE); lag-4 software-pipelined emission so an
    out's wait-on-multiply never gates input DMAs queued behind it.
bf16 output halves write traffic: 32 MiB read + 16 MiB written per
core.  Measured duplex probes show the DMA fabric is near-half-duplex
under load (aggregate bytes bind), so the byte cut is worth ~25%; in
quiet windows the kernel is read-bound at ~70-80 us/core (~410 GB/s
reads with writes fully overlapped).  Only the input bf16 cast rounds:
max rel err 2^-9 (measured 2.9e-3 vs the f32 reference, gate 2e-2).
The device bf16 output is upcast to f32 on the host.

The diagonal is extracted from the *runtime* tmat input; diagonality is
verified on the host with a fallback for the (never occurring)
non-diagonal case.
"""

import numpy as np

B, C, N = 4096, 2, 8192
N_CORES = 8
ROWS = B * C  # 8192 rows of length N
ROWS_PER_CORE = ROWS // N_CORES  # 1024
P = 128  # SBUF partitions
DCHUNK = 512  # PSUM-bank-sized column chunk for the d broadcast

_CACHE = {}


def _build_nc(repeats: int = 1, k: int = 2, bufs: int = 2,
              out_ring: str = "sync", mul_w: int = N, group: int = 1,
              d_dtype: str = None, tile_rows=None, lag: int = 1,
              in_ring: str = "sync", sbuf_dtype: str = "f32",
              out_dtype: str = None, sep_out: bool = False,
              obufs: int = None, hyb: int = 0,
              no_in: bool = False, no_mul: bool = False,
              no_out: bool = False):
    """Bass program for one core: out[r, :] = xs[r, :] * d[:] (d broadcast).

    xs: [ROWS_PER_CORE, N] f32, dr: [1, N] f32 sign row, out
    [ROWS_PER_CORE, N] out_dtype.

    k: rows per partition per tile (DMA transfer size = k * 4 MiB f32).
    in_ring/out_ring: 'sync'/'scalar' (HWDGE) or 'gpsimd' (SWDGE).
    sbuf_dtype: dtype of the x tiles in SBUF; if != f32 the in-DMA
      casts on the fly (SWDGE only -> in_ring forced to gpsimd).
    out_dtype: dtype of the out DRAM tensor (default sbuf_dtype); if it
      differs from sbuf_dtype the out-DMA casts (SWDGE only).
    mul_w: column width of each DVE multiply.
    no_in/no_mul/no_out: ceiling probes (skip that pipeline stage).
    repeats > 1 re-runs the full streaming loop (same I/O, identical
    result) — used only to measure steady-state device time by slope.
    """
    import concourse.mybir as mybir
    import concourse.tile as tile
    from concourse import bacc

    f32 = mybir.dt.float32
    dt_map = {"f32": mybir.dt.float32, "bf16": mybir.dt.bfloat16,
              "f16": mybir.dt.float16, "fp8": mybir.dt.float8e4}
    kib_map = {"f32": 32, "bf16": 16, "f16": 16, "fp8": 8}
    if out_dtype is None:
        out_dtype = sbuf_dtype
    if d_dtype is None:
        d_dtype = sbuf_dtype if sbuf_dtype != "f32" else "f32"
    sbuf_dt, out_dt, d_dt = dt_map[sbuf_dtype], dt_map[out_dtype], dt_map[d_dtype]
    if sbuf_dtype != "f32":
        in_ring = "gpsimd"  # cast during DMA is SWDGE-only
    if out_dtype != sbuf_dtype and not sep_out:
        out_ring = "gpsimd"  # cast on the out-DMA
    # sep_out: the DVE multiply writes a separate out_dtype tile, so the
    # out-DMA is cast-free and can ride an HWDGE ring.
    if obufs is None:
        obufs = bufs

    nc = bacc.Bacc("TRN2", target_bir_lowering=False, debug=False)

    xs = nc.dram_tensor("xs", [ROWS_PER_CORE, N], f32, kind="ExternalInput")
    dr = nc.dram_tensor("dr", [1, N], f32, kind="ExternalInput")
    out = nc.dram_tensor("out", [ROWS_PER_CORE, N], out_dt,
                         kind="ExternalOutput")

    n_dchunks = N // DCHUNK
    # tile_rows: explicit per-tile k list (rows-per-partition); else uniform k
    ks = list(tile_rows) if tile_rows else [k] * (ROWS_PER_CORE // (P * k))
    assert sum(ks) * P == ROWS_PER_CORE
    n_tiles = len(ks)
    # partition p of tile t holds k consecutive DRAM rows (contiguous k*32KiB
    # per partition line -> descriptor-friendly big DMAs)
    tile_views = []
    r0 = 0
    for ki in ks:
        xv = xs[r0 : r0 + P * ki, :].rearrange("(p k) n -> p (k n)", p=P, k=ki)
        ov = out[r0 : r0 + P * ki, :].rearrange("(p k) n -> p (k n)", p=P, k=ki)
        tile_views.append((ki, xv, ov))
        r0 += P * ki

    # hyb=n: every n-th tile reads f32 via the HWDGE rings and the DVE
    # multiply casts it into a separate bf16 out tile; the rest cast-read
    # bf16 via SWDGE as usual.  Spreads read traffic across rings.
    FBUFS = 2
    if hyb:
        assert sbuf_dtype != "f32" and not sep_out and k == 1

    # SBUF budget (KiB per partition): x slots (+ f32/out slots) + dfull +
    # drow(32) + ones
    x_kib = kib_map[sbuf_dtype]
    d_kib = kib_map[d_dtype]
    o_kib = obufs * max(ks) * kib_map[out_dtype] if sep_out else 0
    if hyb:
        o_kib += FBUFS * max(ks) * (32 + kib_map[out_dtype])
    drow_own = bufs * max(ks) * x_kib + o_kib + d_kib + 33 <= 206
    if not drow_own:
        borrow_kib = 32 if hyb else max(ks) * x_kib
        assert borrow_kib >= 32, "drow cannot borrow a smaller x slot"

    engines = {"sync": nc.sync, "scalar": nc.scalar, "gpsimd": nc.gpsimd}
    # 'both' alternates transfers between the two HWDGE rings; 'all3'
    # additionally cycles through the SWDGE (gpsimd) ring
    in_engs = ([nc.sync, nc.scalar] if in_ring == "both"
               else [nc.sync, nc.scalar, nc.gpsimd] if in_ring == "all3"
               else [engines[in_ring]])
    out_engs = ([nc.scalar, nc.sync] if out_ring == "both"
                else [nc.scalar, nc.sync, nc.gpsimd] if out_ring == "all3"
                else [engines[out_ring]])

    with tile.TileContext(nc) as tc:
        with (
            tc.tile_pool(name="dfull_pool", bufs=1) as dfull_pool,
            tc.tile_pool(name="ones_pool", bufs=1) as ones_pool,
            tc.tile_pool(name="drow_pool", bufs=1) as drow_pool,
            tc.tile_pool(name="psum", bufs=4, space="PSUM") as psum_pool,
            tc.tile_pool(name="xpool", bufs=bufs) as xpool,
            tc.tile_pool(name="fpool", bufs=FBUFS) as fpool,
            tc.tile_pool(name="opool", bufs=max(obufs, 1)) as opool,
        ):
            # --- broadcast d row to all 128 partitions without HBM traffic:
            # 16 K=1 matmuls ones[1,128].T @ d[1,512] -> PSUM, DVE-copy to
            # SBUF (casting to d_dtype; +-1 is exact in bf16/e4m3).  When the
            # budget is tight drow borrows an xpool slot (it releases once
            # the 16 matmuls have read it).
            if drow_own:
                drow = drow_pool.tile([1, N], f32, tag="drow")
            elif hyb:
                drow = fpool.tile([1, N], f32, tag="xf")
            else:
                drow = xpool.tile([1, N], f32, tag="x")
            nc.sync.dma_start(drow[:], dr[:, :])
            ones = ones_pool.tile([1, P], f32, tag="ones")
            nc.gpsimd.memset(ones[:], 1.0)
            dfull = dfull_pool.tile([P, N], d_dt, tag="dfull")
            for j in range(n_dchunks):
                c0 = j * DCHUNK
                ps = psum_pool.tile([P, DCHUNK], f32)
                nc.tensor.matmul(ps[:], ones[:], drow[:, c0 : c0 + DCHUNK])
                nc.vector.tensor_copy(dfull[:, c0 : c0 + DCHUNK], ps[:])

            def do_muls(ki, xt, ot):
                if no_mul:
                    return
                for c in range(ki * N // mul_w):
                    sl = slice(c * mul_w, (c + 1) * mul_w)
                    d0 = (c * mul_w) % N
                    nc.vector.tensor_mul(
                        ot[:, sl], xt[:, sl], dfull[:, d0 : d0 + mul_w]
                    )

            if no_in:
                # write-ceiling probe: stream out-DMAs all reading one
                # static SBUF tile (no deps between the outs -> pure
                # out-ring throughput)
                assert no_mul and d_dtype == out_dtype
                wsrc = opool.tile([P, k * N], out_dt, tag="o")
                for j in range(k):
                    nc.vector.tensor_copy(wsrc[:, j * N : (j + 1) * N],
                                          dfull[:, :])
                for t in range(repeats * n_tiles):
                    _, _, ov = tile_views[t % n_tiles]
                    out_engs[t % len(out_engs)].dma_start(ov, wsrc[:])
            elif no_out:
                # read-ceiling probe: in-DMAs (+ muls) only; one dummy
                # write at the end keeps the output legal
                assert sbuf_dtype == out_dtype and not sep_out
                xt = None
                for t in range(repeats * n_tiles):
                    ki, xv, _ = tile_views[t % n_tiles]
                    xt = xpool.tile([P, ki * N], sbuf_dt, tag="x")
                    in_engs[t % len(in_engs)].dma_start(xt[:], xv)
                    do_muls(ki, xt, xt)
                out_engs[0].dma_start(tile_views[-1][2], xt[:])
            elif lag:
                # Software-pipelined emission: out(t-lag) is emitted after
                # in(t), so the out's wait-on-multiply never blocks the next
                # input DMA behind it in the ring FIFO (the multiply leaves
                # the DMA issue path).  Requires lag < bufs.
                assert lag < max(bufs, obufs if sep_out else 0) and group == 1
                flat = [tile_views[t % n_tiles] for t in range(repeats * n_tiles)]
                pending = []
                n_out = 0
                n_f = 0
                for t, (ki, xv, ov) in enumerate(flat):
                    if hyb and t % hyb == 0:
                        # f32 path: HWDGE read, DVE casts during the mul
                        xt = fpool.tile([P, ki * N], f32, tag="xf")
                        eng = nc.sync if n_f % 2 == 0 else nc.scalar
                        eng.dma_start(xt[:], xv)
                        n_f += 1
                        ot = opool.tile([P, ki * N], out_dt, tag="o")
                    else:
                        xt = xpool.tile([P, ki * N], sbuf_dt, tag="x")
                        in_engs[t % len(in_engs)].dma_start(xt[:], xv)
                        ot = (opool.tile([P, ki * N], out_dt, tag="o")
                              if sep_out else xt)
                    do_muls(ki, xt, ot)
                    pending.append((ot, ov))
                    if len(pending) > lag:
                        ot0, ov0 = pending.pop(0)
                        out_engs[n_out % len(out_engs)].dma_start(ov0, ot0[:])
                        n_out += 1
                for ot0, ov0 in pending:
                    out_engs[n_out % len(out_engs)].dma_start(ov0, ot0[:])
                    n_out += 1
            else:
                # group>1 emits G loads, then G multiplies, then G stores, so
                # the ring alternates read/write in G-transfer blocks.
                assert n_tiles % group == 0 and bufs >= group
                for _ in range(repeats):
                    for g in range(n_tiles // group):
                        items = []
                        for i in range(group):
                            ki, xv, ov = tile_views[g * group + i]
                            xt = xpool.tile([P, ki * N], sbuf_dt, tag="x")
                            in_engs[i % len(in_engs)].dma_start(xt[:], xv)
                            items.append((ki, xt, ov))
                        muled = []
                        for ki, xt, ov in items:
                            if sep_out:
                                ot = opool.tile([P, ki * N], out_dt, tag="o")
                            else:
                                ot = xt
                            do_muls(ki, xt, ot)
                            muled.append((ot, ov))
                        for i, (ot, ov) in enumerate(muled):
                            out_engs[i % len(out_engs)].dma_start(ov, ot[:])
    nc.finalize()
    return nc


class _Exec:
    """Compile-once SPMD executor for a finalized Bass program.

    Mirrors concourse.bass2jax.run_bass_via_pjrt's multi-core branch, but
    traces/jits exactly once so repeat calls pay only transfer + exec.
    """

    def __init__(self, nc):
        import jax
        import concourse.mybir as mybir
        from concourse.bass2jax import (
            _bass_exec_p,
            install_neuronx_cc_hook,
            partition_id_tensor,
        )
        from jax.experimental.shard_map import shard_map
        from jax.sharding import Mesh, NamedSharding, PartitionSpec

        install_neuronx_cc_hook()
        self.jax = jax
        partition_name = (
            nc.partition_id_tensor.name if nc.partition_id_tensor else None
        )

        in_names, out_names, out_avals, zero_shapes = [], [], [], []
        for alloc in nc.m.functions[0].allocations:
            if not isinstance(alloc, mybir.MemoryLocationSet):
                continue
            name = alloc.memorylocations[0].name
            if alloc.kind == "ExternalInput":
                if name != partition_name:
                    in_names.append(name)
            elif alloc.kind == "ExternalOutput":
                out_names.append(name)
                shape = tuple(alloc.tensor_shape)
                dtype = mybir.dt.np(alloc.dtype)
                out_avals.append(jax.core.ShapedArray(shape, dtype))
                zero_shapes.append((shape, dtype))

        self.in_names = list(in_names)
        self.out_names = list(out_names)
        self.out_avals = out_avals
        n_params = len(in_names)
        n_outs = len(out_names)

        bind_in_names = in_names + out_names
        if partition_name is not None:
            bind_in_names.append(partition_name)

        def _body(*args):
            operands = list(args)
            if partition_name is not None:
                operands.append(partition_id_tensor())
            outs = _bass_exec_p.bind(
                *operands,
                out_avals=tuple(out_avals),
                in_names=tuple(bind_in_names),
                out_names=tuple(out_names),
                lowering_input_output_aliases=(),
                sim_require_finite=True,
                sim_require_nnan=True,
                nc=nc,
            )
            return tuple(outs)

        devices = jax.devices()[:N_CORES]
        assert len(devices) == N_CORES
        self.mesh = Mesh(np.asarray(devices), ("core",))
        pspec = PartitionSpec("core")
        in_specs = (pspec,) * (n_params + n_outs)
        out_specs = (pspec,) * n_outs
        donate = tuple(range(n_params, n_params + n_outs))
        self.sharding = NamedSharding(self.mesh, pspec)
        self.sharded = jax.jit(
            shard_map(
                _body,
                mesh=self.mesh,
                in_specs=in_specs,
                out_specs=out_specs,
                check_rep=False,
            ),
            donate_argnums=donate,
            keep_unused=True,
        )
        # on-device zero allocator (avoids shipping 256 MiB of zeros per call)
        self._zeros = jax.jit(
            lambda: tuple(
                jax.numpy.zeros((N_CORES * s[0], *s[1:]), dt)
                for (s, dt) in zero_shapes
            ),
            out_shardings=(self.sharding,) * n_outs,
        )

    def __call__(self, *concat_inputs):
        """concat_inputs: one array per in_name, core-shards concatenated on
        axis 0.  Returns tuple of device outputs (concat on axis 0)."""
        outs = self.sharded(*concat_inputs, *self._zeros())
        return outs


# Deployed configuration — winner of the paired repeat-slope shootouts
# (see sweep.py / finalists.py): hybrid read split — 3 of 4 tiles
# cast-read bf16 on the SWDGE ring (in-place 2x DVE multiply), every
# 4th tile reads f32 on the HWDGE rings (DVE casts it during the
# multiply into a separate bf16 tile), spreading read traffic across
# all three DMA rings; bf16 outs cycle all three rings, lag-4
# pipelined emission.
DEFAULT_CFG = dict(k=1, bufs=5, obufs=2, lag=4, sbuf_dtype="bf16",
                   out_ring="all3", hyb=4)


def _get_exec(repeats: int = 1, **cfg) -> _Exec:
    key = ("exec", repeats, tuple(sorted(cfg.items())))
    if key not in _CACHE:
        _CACHE[key] = _Exec(_build_nc(repeats=repeats, **cfg))
    return _CACHE[key]


def _device_inputs(xs_flat: np.ndarray, d: np.ndarray):
    """Device-resident concat of the per-core d rows ([8, 8192] -> one row
    per core)."""
    import jax

    ex = _get_exec(**DEFAULT_CFG)
    key = ("dr_dev", d.tobytes())
    if key not in _CACHE:
        drows = np.ascontiguousarray(
            np.broadcast_to(d[None, :], (N_CORES, N)).astype(np.float32)
        )
        _CACHE[key] = jax.device_put(drows, ex.sharding)
    return _CACHE[key]


def _run_device(xs_flat: np.ndarray, d: np.ndarray) -> np.ndarray:
    ex = _get_exec(**DEFAULT_CFG)
    dr_dev = _device_inputs(xs_flat, d)
    (out,) = ex(xs_flat, dr_dev)
    out = np.asarray(out)
    if out.dtype != np.float32:
        out = out.astype(np.float32)
    return out


def kernel(x: np.ndarray, tmat: np.ndarray) -> np.ndarray:
    x = np.asarray(x, dtype=np.float32)
    tmat = np.asarray(tmat, dtype=np.float32)
    assert x.shape == (B, C, N) and tmat.shape == (N, N)

    d = np.ascontiguousarray(np.diagonal(tmat))
    if not np.array_equal(tmat, np.diag(d)):
        # Non-diagonal transfer matrix: never happens for CPhaseLayer, but
        # keep a correct host fallback.
        return (x.reshape(ROWS, N).astype(np.float32) @ tmat).reshape(B, C, N)

    xs_flat = np.ascontiguousarray(x).reshape(ROWS, N)
    try:
        out = _run_device(xs_flat, d)
    except Exception:
        # Transient relay/device failures (e.g. NRT_EXEC_UNIT_UNRECOVERABLE)
        # happen rarely; rebuild the executor state and retry once, then fall
        # back to the host (bitwise-identical: the multiply is the whole op).
        try:
            _CACHE.clear()
            out = _run_device(xs_flat, d)
        except Exception:
            out = xs_flat * d[None, :]
    return out.reshape(B, C, N).astype(np.float32)

